# revision 57
# baseline (speedup 1.0000x reference)
"""DualReprogrammingLayer Trainium2 kernel.

Sharding: 2 blocks (trend/detail) x 4 head-groups (4 heads each).
Each core computes, for ALL 4096 rows and its 4 heads of ONE block:
  KT = (Wk.T @ protoT)           (heads-slice, S)        [K-proj, bf16 out]
  V  = (protoT.T @ Wv) + bv      (S, heads-slice)        [V-proj]
  qT = (Wq.T @ xT) + bq          (heads-slice, rows)     [bf16]
  scoresT = KT_h @ qT_h          (S, rows) per head      [bf16, 2-head row-packed]
  P  = exp(scoresT / 8)                                  [ACT, bf16 out]
  A_ext = [V_h | ones].T @ P     (64+64, rows);  rows 64:128 = denom
  A_n = A / denom                                        [bf16]
  out partial = A_n.T-stack @ Wo                         [bf16]
  gate = sigmoid(relu(xg_cat @ W1) @ W2)  for 512 OWN rows only
Host: per block, sum the 4 head-group partials, assemble the full gate
from the 8 cores' 512-row pieces, and combine
  out = g * (P_t + bo_t) + (1-g) * (P_d + bo_d).
"""
import sys
sys.path.insert(0, '/opt/trn_rl_repo')
from contextlib import ExitStack

import numpy as np
import ml_dtypes

import concourse.bass as bass
import concourse.tile as tile
from concourse import bacc, mybir

F32 = mybir.dt.float32
BF16 = mybir.dt.bfloat16
AF = mybir.ActivationFunctionType
bf16 = ml_dtypes.bfloat16

B, L, D, S, DLLM, H, E = 4, 1024, 1024, 1000, 4096, 16, 64
HG = 4                        # head-groups per block; cores = 2 blocks x HG
R = B * L                     # 4096 rows per core (all rows)
NH = H // HG                  # 4 heads per core
HEC = NH * E                  # 256
SCH, NSC = 125, 8             # S = 8 chunks of 125
RC, NRC = 512, 8              # rows = 8 chunks of 512
GR = 512                      # gate rows per core
KD = D // 128                 # 8 k-chunks for d_model
KL = DLLM // 128              # 32 k-chunks for d_llm

_CACHE = {}
LAST_RESULTS = None           # set by kernel(): BassKernelResults


def _build():
    nc = bacc.Bacc("TRN2", target_bir_lowering=False, debug=False)

    def din(name, shape, dt):
        return nc.dram_tensor(name, list(shape), dt, kind="ExternalInput")

    # all (128, c, m) tensors are pre-transposed on the host so every DMA
    # reads contiguous per-partition lines (strided gathers run ~10x slower)
    xT = din("xT", (128, KD, R), BF16)
    xgT = {b: din(f"xgT_{b}", (128, KD, GR), BF16) for b in "td"}
    pT = din("pT", (DLLM, S), BF16)
    wq = din("wq", (128, KD, HEC), BF16)
    wk = din("wk", (128, KL, HEC), BF16)
    wv = din("wv", (128, KL, HEC), BF16)
    wo = din("wo", (128, 2, DLLM), BF16)             # [mc0, mc1]
    w1 = din("w1", (128, 2 * KD, D), BF16)
    w2 = din("w2", (128, KD, 1), BF16)
    bq2 = din("bq2", (128, 2), F32)                  # cols: mc0, mc1
    bk2 = din("bk2", (128, 2), F32)
    bvv = din("bv", (1, HEC), BF16)
    gb1 = din("gb1", (128, KD), F32)
    gb2 = din("gb2", (1, 1), F32)
    out = nc.dram_tensor("out", [R, DLLM], BF16, kind="ExternalOutput")
    gate_out = nc.dram_tensor("gate", [1, GR], F32, kind="ExternalOutput")

    with tile.TileContext(nc) as tc, ExitStack() as ctx:
        # ---- persistent pools (live across phases) ----
        pers = ctx.enter_context(tc.tile_pool(name="pers", bufs=1))
        kt_sb = pers.tile([128, 2, S], BF16, tag="kt")      # HE chunk mc at [:, mc, :]
        vx_sb = pers.tile([SCH, NSC, NH, 65], BF16, tag="vx")  # [V_h | ones]
        qt_sb = pers.tile([128, 2, R], BF16, tag="qt")
        ones125 = pers.tile([1, SCH], BF16, tag="ones125")
        nc.vector.memset(ones125[:], 1.0)
        bq_sb = pers.tile([128, 2], F32, tag="bq")
        nc.gpsimd.dma_start(bq_sb[:], bq2.ap())
        bk_sb = pers.tile([128, 2], F32, tag="bk")
        nc.gpsimd.dma_start(bk_sb[:], bk2.ap())
        bv_sb = pers.tile([1, HEC], BF16, tag="bv")
        nc.gpsimd.dma_start(bv_sb[:], bvv.ap())
        gb1_sb = pers.tile([128, KD], F32, tag="gb1")
        nc.gpsimd.dma_start(gb1_sb[:], gb1.ap())
        gb2_sb = pers.tile([1, 1], F32, tag="gb2")
        nc.gpsimd.dma_start(gb2_sb[:], gb2.ap())

        # ---- prefetch pools: weights/x for later phases, loaded during A.
        p_pre2 = ctx.enter_context(tc.tile_pool(name="p_pre2", bufs=1))
        wo_t = p_pre2.tile([128, 2, DLLM], BF16, tag="wo")
        pre1ctx = ExitStack()
        p_pre1 = pre1ctx.enter_context(tc.tile_pool(name="p_pre1", bufs=1))
        w1_t = p_pre1.tile([128, 2 * KD, D], BF16, tag="w1")
        w2_t = p_pre1.tile([128, KD, 1], BF16, tag="w2")
        wq_t = p_pre1.tile([128, KD, HEC], BF16, tag="wq")
        xg_t = {}
        for b in "td":
            xg_t[b] = p_pre1.tile([128, KD, GR], BF16, tag=f"xg_{b}",
                                  name=f"xg_{b}")
        prexctx = ExitStack()
        p_prex = prexctx.enter_context(tc.tile_pool(name="p_prex", bufs=1))
        xt0 = p_prex.tile([128, KD, RC], BF16, tag="xt0")

        # ---- phase A: K/V projections (proto and weights streamed per k-chunk) ----
        with ExitStack() as actx:
            p_pt = actx.enter_context(tc.tile_pool(name="p_pt", bufs=8))
            p_wc = actx.enter_context(tc.tile_pool(name="p_wc", bufs=2))
            psA = actx.enter_context(tc.tile_pool(name="psA", bufs=1, space="PSUM"))
            # PE warm-up: dummy matmuls on memset tiles so the HAM clock-gate
            # ramps while the initial DMAs land. Results are discarded.
            wu_l = p_wc.tile([64, 128], BF16, tag="wu_l")
            nc.vector.memset(wu_l[:], 0.0)
            wu_r = p_wc.tile([64, 512], BF16, tag="wu_r")
            nc.vector.memset(wu_r[:], 0.0)

            # In-flight DMA transfers share the ~350GB/s aggregate roughly
            # equally, so the first-needed transfer crawls if the queues are
            # flooded at t=0. Keep the initial in-flight set tiny: 4 pt
            # chunks + quarter-0's first half; everything later is paced by
            # queue FIFO position and WAR deps on the 2-deep quarter ring.
            # (dma_starts reach their queue at sequencer time, nearly all at
            # t~0 -- FIFO position and data deps are the ONLY real pacing.)
            QK4 = KL // 4
            wkq = [p_wc.tile([128, QK4, HEC], BF16, tag=f"wk_q{q}",
                             name=f"wk_{q}") for q in range(2)]
            wvq = [p_wc.tile([128, QK4, HEC], BF16, tag=f"wv_q{q}",
                             name=f"wv_{q}") for q in range(2)]

            def post_quarter(q):
                hq = slice(q * QK4, (q + 1) * QK4)
                nc.scalar.dma_start(wvq[q % 2][:], wv.ap()[:, hq, :])
                nc.scalar.dma_start(wkq[q % 2][:], wk.ap()[:, hq, :])

            # pre-issue the first 6 pt chunks across all three queues AHEAD
            # of the quarter posts (scalar FIFO: pt2, pt5, q0a, ...) so the
            # critical early stream isn't behind 2MB of weights
            pt_r = pT.ap().rearrange("(c p) s -> c p s", c=KL)
            pt_pre = []
            for kc in range(6):
                ptt = p_pt.tile([128, S], BF16, tag="pt")
                (nc.sync, nc.gpsimd, nc.scalar)[kc % 3].dma_start(
                    ptt[:], pt_r[kc])
                pt_pre.append(ptt)
            for h in range(2):
                hq = slice(h * QK4 // 2, (h + 1) * QK4 // 2)
                nc.scalar.dma_start(wvq[0][:, hq, :], wv.ap()[:, hq, :])
                nc.scalar.dma_start(wkq[0][:, hq, :], wk.ap()[:, hq, :])
            post_quarter(1)

            vps = [psA.tile([SCH, 2, HEC], F32, tag=f"vps{i}", name=f"vps{i}")
                   for i in range(4)]
            kps = [psA.tile([128, 512], F32, tag=f"kps{i}", name=f"kps{i}")
                   for i in range(4)]
            # discarded warm-up group in kps[0]'s bank, closed before the
            # real accumulation group opens
            for wi in range(8):
                nc.tensor.matmul(kps[0][:], wu_l[:], wu_r[:],
                                 start=(wi == 0), stop=(wi == 7))
            def emit_k(kcK, ptk):
                for mc in range(2):
                    for ncc in range(2):
                        nc.tensor.matmul(
                            kps[mc * 2 + ncc][:, 0:500],
                            wkq[(kcK // QK4) % 2][:, kcK % QK4,
                                                  mc * 128:(mc + 1) * 128],
                            ptk[:, ncc * 500:(ncc + 1) * 500],
                            start=(kcK == 0), stop=(kcK == KL - 1))

            # K-proj lags V-proj by 2 chunks (same total work, accumulation
            # groups unaffected): its wk deadline shifts ~2us later, and the
            # lagged K chunks -- emitted ahead of V in each iteration --
            # absorb pt-supply stalls since their inputs landed long ago
            KLAG = 2
            pt_tiles = {}
            for kc in range(KL):
                if kc < 6:
                    pt_t = pt_pre[kc]
                else:
                    pt_t = p_pt.tile([128, S], BF16, tag="pt")
                    # alternate queues: a single queue can't sustain the
                    # ~2us/chunk demand and accumulates multi-us stalls
                    (nc.sync if kc % 2 == 0 else nc.gpsimd).dma_start(
                        pt_t[:], pt_r[kc])
                pt_tiles[kc] = pt_t
                # ring re-loads must be EMITTED after the old slot's last
                # reader (K(7) at iteration 9, K(15) at 17) or they'd alias
                # the pending reads
                if kc == 10:
                    post_quarter(2)
                elif kc == 18:
                    post_quarter(3)
                if kc >= KLAG:
                    emit_k(kc - KLAG, pt_tiles[kc - KLAG])
                wvc = wvq[(kc // QK4) % 2][:, kc % QK4, :]
                for si in range(NSC):
                    # one accumulation group per PSUM bank: only the first
                    # half issues start=True (bank-wide clear covers both)
                    nc.tensor.matmul(
                        vps[si // 2][:, si % 2, :],
                        pt_t[:, si * SCH:(si + 1) * SCH],
                        wvc,
                        start=(kc == 0 and si % 2 == 0), stop=False)
            for kcK in range(KL - KLAG, KL):
                emit_k(kcK, pt_tiles[kcK])
            # behind the full pt streams on sync/gpsimd (balanced): these
            # land 50-70us, needed at 100us+; keeping them off the scalar
            # queue matters because phase A saturates the DMA engines and
            # scalar already carries the 4MB of wk/wv quarters
            nc.sync.dma_start(wq_t[:], wq.ap())
            nc.gpsimd.dma_start(xt0[:], xT.ap()[:, :, 0:RC])
            nc.gpsimd.dma_start(xg_t["t"][:], xgT["t"].ap())
            nc.sync.dma_start(xg_t["d"][:], xgT["d"].ap())
            for si in range(NSC):
                nc.tensor.matmul(
                    vps[si // 2][:, si % 2, :],
                    ones125[:],
                    bv_sb[:],
                    start=False, stop=(si % 2 == 1))
            for si in range(NSC):
                # copy V psum (125, 256) -> [:, si, :, 0:64] viewed as (125, 4, 64)
                nc.vector.tensor_copy(
                    vx_sb[:, si, :, 0:64],
                    vps[si // 2][:, si % 2, :].rearrange("p (h e) -> p h e", h=NH))
            nc.vector.memset(vx_sb[:, :, :, 64:65], 1.0)
            for mc in range(2):
                for ncc in range(2):
                    nc.scalar.activation(
                        kt_sb[:, mc, ncc * 500:(ncc + 1) * 500],
                        kps[mc * 2 + ncc][:, 0:500],
                        AF.Identity,
                        bias=bk_sb[:, mc:mc + 1])

        # ---- phase B: Q projection for rows-chunk 0 (rest are C fillers) ----
        with ExitStack() as bctx:
            psB = bctx.enter_context(tc.tile_pool(name="psB", bufs=2, space="PSUM"))
            # the (now idle) sync queue prefetches phase-C weights in
            # need-order: w1/w2 (gate filler in early C), then wo. w1 in the
            # phase-A chain instead starves the pt stream for ~10us.
            nc.sync.dma_start(w1_t[:], w1.ap())
            nc.sync.dma_start(w2_t[:], w2.ap())
            nc.sync.dma_start(wo_t[:], wo.ap())
            for mc in range(2):
                qps = psB.tile([128, RC], F32, tag="qps")
                for kc in range(KD):
                    nc.tensor.matmul(
                        qps[:],
                        wq_t[:, kc, mc * 128:(mc + 1) * 128],
                        xt0[:, kc, :],
                        start=(kc == 0), stop=(kc == KD - 1))
                nc.scalar.activation(
                    qt_sb[:, mc, 0:RC], qps[:], AF.Identity,
                    bias=bq_sb[:, mc:mc + 1])

        prexctx.close()

        # ---- phase C: attention + output projection ----
        # Software pipeline: QK+exp of unit u overlaps PV of unit u-1 at
        # s-chunk granularity. Units are the two mc halves; fillers per chunk
        # are the previous chunk's out-projection, the next chunk's Q
        # projection, and (chunks 0-1) the gate MLP.
        with ExitStack() as cctx:
            p_p = cctx.enter_context(tc.tile_pool(name="p_p", bufs=5))
            p_a = cctx.enter_context(tc.tile_pool(name="p_a", bufs=2))
            p_s = cctx.enter_context(tc.tile_pool(name="p_s", bufs=2))
            p_o = cctx.enter_context(tc.tile_pool(name="p_o", bufs=4))
            psS = cctx.enter_context(tc.tile_pool(name="psS", bufs=1, space="PSUM"))
            psPV = cctx.enter_context(tc.tile_pool(name="psPV", bufs=1, space="PSUM"))
            psO = cctx.enter_context(tc.tile_pool(name="psO", bufs=2, space="PSUM"))
            p_cx = cctx.enter_context(tc.tile_pool(name="p_cx", bufs=1))
            p_x2 = cctx.enter_context(tc.tile_pool(name="p_x2", bufs=2))

            def emit_qk_exp(mc, si, rsl, rc):
                # NOTE: both packed matmuls must target the SAME psum tile --
                # per-head psum tiles serialize the tile_position pair
                # (measured +136ns per QK pair)
                sps2 = psS.tile([SCH, 2, RC], F32, tag="sps", name="sps")
                for hh in range(2):  # row-packed pair, adjacent emission
                    po = hh * 64
                    nc.tensor.matmul(
                        sps2[:, hh, 0:rc],
                        kt_sb[po:po + 64, mc, si * SCH:(si + 1) * SCH],
                        qt_sb[po:po + 64, mc, rsl],
                        start=True, stop=True,
                        tile_position=(po, 0))
                p2 = p_p.tile([SCH, 2, RC], BF16, tag=f"p{si % 2}",
                              name=f"p{si % 2}")
                nc.scalar.activation(p2[:, :, 0:rc], sps2[:, :, 0:rc],
                                     AF.Exp, scale=0.125)
                return p2

            def emit_pv(aps, mc, si, p2, rc):
                for hh in range(2):
                    h = mc * 2 + hh
                    nc.tensor.matmul(
                        aps[hh][:, 0:rc], vx_sb[:, si, h, :],
                        p2[:, hh, 0:rc],
                        start=(si == 0), stop=(si == NSC - 1))

            def emit_norm(aps, mc, a2, rc):
                for hh in range(2):
                    den1 = p_s.tile([1, RC], F32, tag="den1")
                    nc.vector.tensor_copy(den1[:, 0:rc], aps[hh][64:65, 0:rc])
                    rec1 = p_s.tile([1, RC], F32, tag="rec1")
                    nc.vector.reciprocal_approx_fast(rec1[:, 0:rc], den1[:, 0:rc])
                    sct64 = p_s.tile([64, RC], F32, tag="sct64")
                    nc.gpsimd.partition_broadcast(sct64[:, 0:rc], rec1[:, 0:rc])
                    nc.vector.tensor_mul(
                        a2[mc][hh * 64:hh * 64 + 64, 0:rc],
                        aps[hh][0:64, 0:rc], sct64[:, 0:rc])

            bstate = {}

            def emit_q_load(row0, rc):
                xtB = p_x2.tile([128, KD, RC], BF16, tag="xB", name="xB")
                nc.gpsimd.dma_start(xtB[:, :, 0:rc],
                                    xT.ap()[:, :, row0:row0 + rc])
                bstate["x"] = xtB

            def emit_q_proj(row0, rc, mcq):
                qps = psO.tile([128, RC], F32, tag="ops", name="qpsB")
                for kc in range(KD):
                    nc.tensor.matmul(
                        qps[:, 0:rc],
                        wq_t[:, kc, mcq * 128:(mcq + 1) * 128],
                        bstate["x"][:, kc, 0:rc],
                        start=(kc == 0), stop=(kc == KD - 1))
                nc.scalar.activation(
                    qt_sb[:, mcq, row0:row0 + rc], qps[:, 0:rc], AF.Identity,
                    bias=bq_sb[:, mcq:mcq + 1])

            def emit_gate_piece(mc):
                # hidden chunk mc for this core's 512 gate rows + its logit
                # partial, accumulated in SBUF (no persistent PSUM bank)
                hps = psO.tile([128, GR], F32, tag="ops", name="hpsG")
                for kc in range(2 * KD):
                    nc.tensor.matmul(
                        hps[:],
                        w1_t[:, kc, mc * 128:(mc + 1) * 128],
                        xg_t["t" if kc < KD else "d"][:, kc % KD, :],
                        start=(kc == 0), stop=(kc == 2 * KD - 1))
                htmp = p_s.tile([128, GR], BF16, tag="htmp", name="htmp")
                nc.scalar.activation(htmp[:], hps[:], AF.Relu,
                                     bias=gb1_sb[:, mc:mc + 1])
                lps = psO.tile([128, GR], F32, tag="ops", name="lpsG")
                nc.tensor.matmul(lps[0:1, :], w2_t[:, mc, :], htmp[:],
                                 start=True, stop=True)
                lacc = bstate["lacc"]
                if mc == 0:
                    nc.vector.tensor_copy(lacc[:], lps[0:1, :])
                else:
                    nc.vector.tensor_add(lacc[:], lacc[:], lps[0:1, :])

            def emit_gate_final():
                gate_sb = p_cx.tile([1, GR], F32, tag="gate", name="gate_sb")
                nc.scalar.activation(gate_sb[:], bstate["lacc"][:],
                                     AF.Sigmoid, bias=gb2_sb[:])
                nc.sync.dma_start(gate_out.ap(), gate_sb[:])

            def emit_gate_init():
                bstate["lacc"] = p_cx.tile([1, GR], F32, tag="lacc", name="lacc")

            def make_outproj_tasks(a2p, row0c, nrb, final=False):
                """Output projection for the rows-chunk at row0c (nrb 128-row
                blocks). One task per PSUM group (2-matmul chain over the two
                mc halves); osb assembled per (rb, half) and DMA'd out."""
                state = {}

                def group(rb, ncc):
                    # spread the 0.5MB output bursts over all three DMA queues
                    dq = (nc.sync, nc.scalar, nc.gpsimd)[(rb * 8 + ncc) % 3
                                                        if final else rb % 3]
                    row0 = row0c + rb * 128
                    half, nh = divmod(ncc, 4)
                    if nh == 0:
                        state[(rb, half)] = p_o.tile([128, DLLM // 2], BF16,
                                                     tag="osb", name="osb")
                    osb = state[(rb, half)]
                    nsl = slice(ncc * 512, (ncc + 1) * 512)
                    ops = psO.tile([128, 512], F32, tag="ops", name="ops")
                    for kk in range(2):
                        nc.tensor.matmul(
                            ops[:], a2p[kk][:, rb * 128:(rb + 1) * 128],
                            wo_t[:, kk, nsl],
                            start=(kk == 0), stop=(kk == 1))
                    if (ncc % 2 == 1) if final else (ncc % 4 == 3):
                        # scalar has some slack next to vector; offload part
                        # of the psum->sbuf copies (half once nothing but the
                        # drain remains)
                        nc.scalar.activation(osb[:, nh * 512:(nh + 1) * 512],
                                             ops[:], AF.Identity)
                    else:
                        nc.vector.tensor_copy(osb[:, nh * 512:(nh + 1) * 512],
                                              ops[:])
                    if final:
                        # drain the last chunk piecewise: 0.25MB per DMA,
                        # issued as soon as each pair of columns is copied
                        if nh % 2 == 1:
                            dq.dma_start(
                                out.ap()[row0:row0 + 128,
                                         (ncc - 1) * 512:(ncc + 1) * 512],
                                osb[:, (nh - 1) * 512:(nh + 1) * 512])
                    elif nh == 3:
                        dq.dma_start(
                            out.ap()[row0:row0 + 128,
                                     half * (DLLM // 2):(half + 1) * (DLLM // 2)],
                            osb[:])

                return [(lambda rb=rb, ncc=ncc: group(rb, ncc))
                        for rb in range(nrb) for ncc in range(8)]

            # last 512 rows split in two 256-row chunks so the trailing
            # (non-overlapped) out-projection of the final chunk halves
            chunks = [(r * RC, RC) for r in range(NRC - 1)]
            chunks += [((NRC - 1) * RC, RC // 2), ((NRC - 1) * RC + RC // 2, RC // 2)]
            qsteps = [(row0c, rc) for row0c, rc in chunks[1:]] + [None]
            pending = None   # (a2, row0, nrb) awaiting out projection
            for ci, (row0c, rc) in enumerate(chunks):
                rsl = slice(row0c, row0c + rc)
                a2 = [p_a.tile([128, RC], BF16, tag=f"a2_{mc}",
                               name=f"a2_{mc}") for mc in range(2)]
                # fillers: dependency-free work (next chunk's Q projection,
                # gate MLP pieces on chunks 0-1) paces from slot 0 and covers
                # the first slots; the prev chunk's out-proj paces from slot
                # 3 because its first group reads a2 written by a norm chain
                # still in flight at chunk start
                # fillers this chunk: next chunk's Q projection (dependency
                # free, so it leads), then the prev chunk's out-proj -- its
                # first group must wait out the in-flight norm of THIS
                # chunk's predecessor, hence the 3-slot pacing delay -- and
                # the gate MLP during chunks 0-1
                tasks = []
                if qsteps[ci] is not None:
                    qr0, qrc = qsteps[ci]
                    tasks += [lambda r=qr0, c=qrc: emit_q_load(r, c),
                              lambda r=qr0, c=qrc: emit_q_proj(r, c, 0),
                              lambda r=qr0, c=qrc: emit_q_proj(r, c, 1)]
                if ci == 0:
                    tasks += [emit_gate_init]
                    tasks += [(lambda m=m: emit_gate_piece(m)) for m in range(4)]
                elif ci == 1:
                    tasks += [(lambda m=m: emit_gate_piece(m)) for m in range(4, 8)]
                    tasks += [emit_gate_final]
                if pending is not None:
                    tasks += make_outproj_tasks(*pending)
                ti = 0
                slot = 0
                nslots = 3 * NSC   # 2 units + drain, at s-chunk granularity
                prev = None        # (aps, mc, p2dict)
                for mc in range(2):
                    aps = [psPV.tile([65, RC], F32, tag=f"aps{mc}{hh}",
                                     name=f"aps{mc}{hh}") for hh in range(2)]
                    p2buf = {}
                    for si in range(NSC):
                        p2buf[si] = emit_qk_exp(mc, si, rsl, rc)
                        if prev is not None:
                            paps, pmc, pp2 = prev
                            emit_pv(paps, pmc, si, pp2[si], rc)
                        slot += 1
                        due = max(0, ((slot - 3) * len(tasks)) // (nslots - 3))
                        while ti < due:
                            tasks[ti]()
                            ti += 1
                    if prev is not None:
                        emit_norm(prev[0], prev[1], a2, rc)
                    prev = (aps, mc, p2buf)
                # drain last unit of this rows-chunk; fillers keep interleaving
                paps, pmc, pp2 = prev
                for si in range(NSC):
                    emit_pv(paps, pmc, si, pp2[si], rc)
                    slot += 1
                    due = max(0, ((slot - 3) * len(tasks)) // (nslots - 3))
                    while ti < due:
                        tasks[ti]()
                        ti += 1
                emit_norm(paps, pmc, a2, rc)
                while ti < len(tasks):
                    tasks[ti]()
                    ti += 1
                pending = (a2, row0c, rc // 128)
            for task in make_outproj_tasks(*pending, final=True):
                task()
        pre1ctx.close()

    nc.compile()
    return nc


def _prep_inputs(inputs):
    """Host-side shard + transpose. Returns in_maps for 8 cores."""
    f32 = np.float32
    t = {k: np.asarray(v) for k, v in inputs.items()}
    x_full = {"t": t["trend_emb"].reshape(B * L, D).astype(f32),
              "d": t["detail_emb"].reshape(B * L, D).astype(f32)}
    pT_full = {"t": np.ascontiguousarray(t["trend_proto"].astype(f32).T).astype(bf16),
               "d": np.ascontiguousarray(t["detail_proto"].astype(f32).T).astype(bf16)}
    W = {("q", "t"): t["t_Wq"], ("q", "d"): t["d_Wq"],
         ("k", "t"): t["t_Wk"], ("k", "d"): t["d_Wk"],
         ("v", "t"): t["t_Wv"], ("v", "d"): t["d_Wv"],
         ("o", "t"): t["t_Wo"], ("o", "d"): t["d_Wo"]}
    bias = {("q", "t"): t["t_bq"], ("q", "d"): t["d_bq"],
            ("k", "t"): t["t_bk"], ("k", "d"): t["d_bk"],
            ("v", "t"): t["t_bv"], ("v", "d"): t["d_bv"]}

    def to_p(a):
        c = a.shape[0] // 128
        return np.ascontiguousarray(
            a.reshape(c, 128, a.shape[1]).transpose(1, 0, 2)).astype(bf16)

    xT_blk = {b: to_p(np.ascontiguousarray(x_full[b].T)) for b in "td"}
    wq_blk, wk_blk, wv_blk, wo_blk, bq_blk, bk_blk, bv_blk = ({} for _ in range(7))
    for b in "td":
        for hg in range(HG):
            hsl = slice(hg * HEC, (hg + 1) * HEC)
            wq_blk[b, hg] = to_p(np.ascontiguousarray(W[("q", b)][:, hsl]))
            wk_blk[b, hg] = to_p(np.ascontiguousarray(W[("k", b)][:, hsl]))
            wv_blk[b, hg] = to_p(np.ascontiguousarray(W[("v", b)][:, hsl]))
            wo_blk[b, hg] = to_p(np.ascontiguousarray(W[("o", b)][hsl, :]))
            bq_blk[b, hg] = np.stack([bias[("q", b)][hsl][0:128],
                                      bias[("q", b)][hsl][128:256]], axis=1).astype(f32)
            bk_blk[b, hg] = np.stack([bias[("k", b)][hsl][0:128],
                                      bias[("k", b)][hsl][128:256]], axis=1).astype(f32)
            bv_blk[b, hg] = bias[("v", b)][hsl][None, :].astype(bf16)
    w1_p = to_p(t["g_W1"])
    w2_p = to_p(t["g_W2"])
    gb1_p = np.ascontiguousarray(t["g_b1"].astype(f32).reshape(KD, 128).T)
    gb2_p = t["g_b2"].astype(f32).reshape(1, 1)

    in_maps = []
    for core in range(8):
        blk = "t" if core < 4 else "d"
        hg = core % HG
        grow = slice(core * GR, (core + 1) * GR)
        m = {"xT": xT_blk[blk], "pT": pT_full[blk],
             "wq": wq_blk[blk, hg], "wk": wk_blk[blk, hg],
             "wv": wv_blk[blk, hg], "wo": wo_blk[blk, hg],
             "bq2": bq_blk[blk, hg], "bk2": bk_blk[blk, hg],
             "bv": bv_blk[blk, hg],
             "w1": w1_p, "w2": w2_p, "gb1": gb1_p, "gb2": gb2_p}
        for b in "td":
            m[f"xgT_{b}"] = to_p(np.ascontiguousarray(x_full[b][grow].T))
        in_maps.append(m)
    return in_maps


def kernel(**inputs):
    global LAST_RESULTS
    import os
    from concourse.bass_utils import run_bass_kernel_spmd

    in_maps = _prep_inputs(inputs)
    if "nc" not in _CACHE:
        _CACHE["nc"] = _build()
    nc = _CACHE["nc"]

    trace = bool(os.environ.get("KERNEL_TRACE"))
    res = run_bass_kernel_spmd(
        nc, in_maps, list(range(8)),
        trace=trace, trace_cores=list(range(8)) if trace else None)
    LAST_RESULTS = res

    t = {k: np.asarray(v) for k, v in inputs.items()}
    bo = {"t": t["t_bo"].astype(np.float32), "d": t["d_bo"].astype(np.float32)}
    g = np.concatenate([res.results[c]["gate"][0]
                        for c in range(8)]).astype(np.float32)[:, None]
    acc = {}
    for bi, b in enumerate("td"):
        a = res.results[bi * HG]["out"].astype(np.float32)
        for hg in range(1, HG):
            a = a + res.results[bi * HG + hg]["out"]
        acc[b] = a + bo[b][None, :]
    out = g * acc["t"] + (1.0 - g) * acc["d"]
    return out.reshape(B, L, DLLM)


# revision 62
# speedup vs baseline: 1.0031x; 1.0031x over previous
"""DualReprogrammingLayer Trainium2 kernel.

Sharding: 2 blocks (trend/detail) x 4 head-groups (4 heads each).
Each core computes, for ALL 4096 rows and its 4 heads of ONE block:
  KT = (Wk.T @ protoT)           (heads-slice, S)        [K-proj, bf16 out]
  V  = (protoT.T @ Wv) + bv      (S, heads-slice)        [V-proj]
  qT = (Wq.T @ xT) + bq          (heads-slice, rows)     [bf16]
  scoresT = KT_h @ qT_h          (S, rows) per head      [bf16, 2-head row-packed]
  P  = exp(scoresT / 8)                                  [ACT, bf16 out]
  A_ext = [V_h | ones].T @ P     (64+64, rows);  rows 64:128 = denom
  A_n = A / denom                                        [bf16]
  out partial = A_n.T-stack @ Wo                         [bf16]
  gate = sigmoid(relu(xg_cat @ W1) @ W2)  for 512 OWN rows only
Host: per block, sum the 4 head-group partials, assemble the full gate
from the 8 cores' 512-row pieces, and combine
  out = g * (P_t + bo_t) + (1-g) * (P_d + bo_d).
"""
import sys
sys.path.insert(0, '/opt/trn_rl_repo')
from contextlib import ExitStack

import numpy as np
import ml_dtypes

import concourse.bass as bass
import concourse.tile as tile
from concourse import bacc, mybir

F32 = mybir.dt.float32
BF16 = mybir.dt.bfloat16
AF = mybir.ActivationFunctionType
bf16 = ml_dtypes.bfloat16

B, L, D, S, DLLM, H, E = 4, 1024, 1024, 1000, 4096, 16, 64
HG = 4                        # head-groups per block; cores = 2 blocks x HG
R = B * L                     # 4096 rows per core (all rows)
NH = H // HG                  # 4 heads per core
HEC = NH * E                  # 256
SCH, NSC = 125, 8             # S = 8 chunks of 125
RC, NRC = 512, 8              # rows = 8 chunks of 512
GR = 512                      # gate rows per core
KD = D // 128                 # 8 k-chunks for d_model
KL = DLLM // 128              # 32 k-chunks for d_llm

_CACHE = {}
LAST_RESULTS = None           # set by kernel(): BassKernelResults


def _build():
    nc = bacc.Bacc("TRN2", target_bir_lowering=False, debug=False)

    def din(name, shape, dt):
        return nc.dram_tensor(name, list(shape), dt, kind="ExternalInput")

    # all (128, c, m) tensors are pre-transposed on the host so every DMA
    # reads contiguous per-partition lines (strided gathers run ~10x slower)
    xT = din("xT", (128, KD, R), BF16)
    xgT = {b: din(f"xgT_{b}", (128, KD, GR), BF16) for b in "td"}
    pT = din("pT", (DLLM, S), BF16)
    wq = din("wq", (128, KD, HEC), BF16)
    wk = din("wk", (128, KL, HEC), BF16)
    wv = din("wv", (128, KL, HEC), BF16)
    wo = din("wo", (128, 2, DLLM), BF16)             # [mc0, mc1]
    w1 = din("w1", (128, 2 * KD, D), BF16)
    w2 = din("w2", (128, KD, 1), BF16)
    bq2 = din("bq2", (128, 2), F32)                  # cols: mc0, mc1
    bk2 = din("bk2", (128, 2), F32)
    bvv = din("bv", (1, HEC), BF16)
    gb1 = din("gb1", (128, KD), F32)
    gb2 = din("gb2", (1, 1), F32)
    out = nc.dram_tensor("out", [R, DLLM], BF16, kind="ExternalOutput")
    gate_out = nc.dram_tensor("gate", [1, GR], F32, kind="ExternalOutput")

    with tile.TileContext(nc) as tc, ExitStack() as ctx:
        # ---- persistent pools (live across phases) ----
        pers = ctx.enter_context(tc.tile_pool(name="pers", bufs=1))
        kt_sb = pers.tile([128, 2, S], BF16, tag="kt")      # HE chunk mc at [:, mc, :]
        vx_sb = pers.tile([SCH, NSC, NH, 65], BF16, tag="vx")  # [V_h | ones]
        qt_sb = pers.tile([128, 2, R], BF16, tag="qt")
        ones125 = pers.tile([1, SCH], BF16, tag="ones125")
        nc.vector.memset(ones125[:], 1.0)
        bq_sb = pers.tile([128, 2], F32, tag="bq")
        nc.gpsimd.dma_start(bq_sb[:], bq2.ap())
        bk_sb = pers.tile([128, 2], F32, tag="bk")
        nc.gpsimd.dma_start(bk_sb[:], bk2.ap())
        bv_sb = pers.tile([1, HEC], BF16, tag="bv")
        nc.gpsimd.dma_start(bv_sb[:], bvv.ap())
        gb1_sb = pers.tile([128, KD], F32, tag="gb1")
        nc.gpsimd.dma_start(gb1_sb[:], gb1.ap())
        gb2_sb = pers.tile([1, 1], F32, tag="gb2")
        nc.gpsimd.dma_start(gb2_sb[:], gb2.ap())

        # ---- prefetch pools: weights/x for later phases, loaded during A.
        p_pre2 = ctx.enter_context(tc.tile_pool(name="p_pre2", bufs=1))
        wo_t = p_pre2.tile([128, 2, DLLM], BF16, tag="wo")
        pre1ctx = ExitStack()
        p_pre1 = pre1ctx.enter_context(tc.tile_pool(name="p_pre1", bufs=1))
        w1_t = p_pre1.tile([128, 2 * KD, D], BF16, tag="w1")
        w2_t = p_pre1.tile([128, KD, 1], BF16, tag="w2")
        wq_t = p_pre1.tile([128, KD, HEC], BF16, tag="wq")
        xg_t = {}
        for b in "td":
            xg_t[b] = p_pre1.tile([128, KD, GR], BF16, tag=f"xg_{b}",
                                  name=f"xg_{b}")
        prexctx = ExitStack()
        p_prex = prexctx.enter_context(tc.tile_pool(name="p_prex", bufs=1))
        xt0 = p_prex.tile([128, KD, RC], BF16, tag="xt0")

        # ---- phase A: K/V projections (proto and weights streamed per k-chunk) ----
        with ExitStack() as actx:
            p_pt = actx.enter_context(tc.tile_pool(name="p_pt", bufs=8))
            p_wc = actx.enter_context(tc.tile_pool(name="p_wc", bufs=2))
            psA = actx.enter_context(tc.tile_pool(name="psA", bufs=1, space="PSUM"))
            # PE warm-up: dummy matmuls on memset tiles so the HAM clock-gate
            # ramps while the initial DMAs land. Results are discarded.
            wu_l = p_wc.tile([64, 128], BF16, tag="wu_l")
            nc.vector.memset(wu_l[:], 0.0)
            wu_r = p_wc.tile([64, 512], BF16, tag="wu_r")
            nc.vector.memset(wu_r[:], 0.0)

            # In-flight DMA transfers share the ~350GB/s aggregate roughly
            # equally, so the first-needed transfer crawls if the queues are
            # flooded at t=0. Keep the initial in-flight set tiny: 4 pt
            # chunks + quarter-0's first half; everything later is paced by
            # queue FIFO position and WAR deps on the 2-deep quarter ring.
            # (dma_starts reach their queue at sequencer time, nearly all at
            # t~0 -- FIFO position and data deps are the ONLY real pacing.)
            QK4 = KL // 4
            wkq = [p_wc.tile([128, QK4, HEC], BF16, tag=f"wk_q{q}",
                             name=f"wk_{q}") for q in range(2)]
            wvq = [p_wc.tile([128, QK4, HEC], BF16, tag=f"wv_q{q}",
                             name=f"wv_{q}") for q in range(2)]

            def post_quarter(q):
                hq = slice(q * QK4, (q + 1) * QK4)
                nc.scalar.dma_start(wvq[q % 2][:], wv.ap()[:, hq, :])
                nc.scalar.dma_start(wkq[q % 2][:], wk.ap()[:, hq, :])

            # pre-issue the first 6 pt chunks across all three queues AHEAD
            # of the quarter posts (scalar FIFO: pt2, pt5, q0a, ...) so the
            # critical early stream isn't behind 2MB of weights
            pt_r = pT.ap().rearrange("(c p) s -> c p s", c=KL)
            pt_pre = []
            for kc in range(6):
                ptt = p_pt.tile([128, S], BF16, tag="pt")
                (nc.sync, nc.gpsimd, nc.scalar)[kc % 3].dma_start(
                    ptt[:], pt_r[kc])
                pt_pre.append(ptt)
            for h in range(2):
                hq = slice(h * QK4 // 2, (h + 1) * QK4 // 2)
                nc.scalar.dma_start(wvq[0][:, hq, :], wv.ap()[:, hq, :])
                nc.scalar.dma_start(wkq[0][:, hq, :], wk.ap()[:, hq, :])
            post_quarter(1)

            vps = [psA.tile([SCH, 2, HEC], F32, tag=f"vps{i}", name=f"vps{i}")
                   for i in range(4)]
            kps = [psA.tile([128, 512], F32, tag=f"kps{i}", name=f"kps{i}")
                   for i in range(4)]
            # discarded warm-up group in kps[0]'s bank, closed before the
            # real accumulation group opens
            for wi in range(8):
                nc.tensor.matmul(kps[0][:], wu_l[:], wu_r[:],
                                 start=(wi == 0), stop=(wi == 7))
            for kc in range(KL):
                if kc < 6:
                    pt_t = pt_pre[kc]
                else:
                    pt_t = p_pt.tile([128, S], BF16, tag="pt")
                    # alternate queues: a single queue can't sustain the
                    # ~2us/chunk demand and accumulates multi-us stalls
                    (nc.sync if kc % 2 == 0 else nc.gpsimd).dma_start(
                        pt_t[:], pt_r[kc])
                # ring re-loads must be EMITTED after the old slot's last
                # reader (kc=7 resp. 15) or they'd alias the pending reads
                if kc == 8:
                    post_quarter(2)
                elif kc == 9:
                    nc.scalar.dma_start(wq_t[:], wq.ap())
                elif kc == 10:
                    nc.scalar.dma_start(xt0[:], xT.ap()[:, :, 0:RC])
                elif kc == 11:
                    for b in "td":
                        nc.scalar.dma_start(xg_t[b][:], xgT[b].ap())
                elif kc == 16:
                    post_quarter(3)
                wvc = wvq[(kc // QK4) % 2][:, kc % QK4, :]
                for si in range(NSC):
                    # one accumulation group per PSUM bank: only the first
                    # half issues start=True (bank-wide clear covers both)
                    nc.tensor.matmul(
                        vps[si // 2][:, si % 2, :],
                        pt_t[:, si * SCH:(si + 1) * SCH],
                        wvc,
                        start=(kc == 0 and si % 2 == 0), stop=False)
                for mc in range(2):
                    for ncc in range(2):
                        nc.tensor.matmul(
                            kps[mc * 2 + ncc][:, 0:500],
                            wkq[(kc // QK4) % 2][:, kc % QK4, mc * 128:(mc + 1) * 128],
                            pt_t[:, ncc * 500:(ncc + 1) * 500],
                            start=(kc == 0), stop=(kc == KL - 1))
            for si in range(NSC):
                nc.tensor.matmul(
                    vps[si // 2][:, si % 2, :],
                    ones125[:],
                    bv_sb[:],
                    start=False, stop=(si % 2 == 1))
            for si in range(NSC):
                # copy V psum (125, 256) -> [:, si, :, 0:64] viewed as (125, 4, 64)
                nc.vector.tensor_copy(
                    vx_sb[:, si, :, 0:64],
                    vps[si // 2][:, si % 2, :].rearrange("p (h e) -> p h e", h=NH))
            nc.vector.memset(vx_sb[:, :, :, 64:65], 1.0)
            for mc in range(2):
                for ncc in range(2):
                    nc.scalar.activation(
                        kt_sb[:, mc, ncc * 500:(ncc + 1) * 500],
                        kps[mc * 2 + ncc][:, 0:500],
                        AF.Identity,
                        bias=bk_sb[:, mc:mc + 1])

        # ---- phase B: Q projection for rows-chunk 0 (rest are C fillers) ----
        with ExitStack() as bctx:
            psB = bctx.enter_context(tc.tile_pool(name="psB", bufs=2, space="PSUM"))
            # the (now idle) sync queue prefetches phase-C weights in
            # need-order: w1/w2 (gate filler in early C), then wo. w1 in the
            # phase-A chain instead starves the pt stream for ~10us.
            nc.sync.dma_start(w1_t[:], w1.ap())
            nc.sync.dma_start(w2_t[:], w2.ap())
            nc.sync.dma_start(wo_t[:], wo.ap())
            for mc in range(2):
                qps = psB.tile([128, RC], F32, tag="qps")
                for kc in range(KD):
                    nc.tensor.matmul(
                        qps[:],
                        wq_t[:, kc, mc * 128:(mc + 1) * 128],
                        xt0[:, kc, :],
                        start=(kc == 0), stop=(kc == KD - 1))
                nc.scalar.activation(
                    qt_sb[:, mc, 0:RC], qps[:], AF.Identity,
                    bias=bq_sb[:, mc:mc + 1])

        prexctx.close()

        # ---- phase C: attention + output projection ----
        # Software pipeline: QK+exp of unit u overlaps PV of unit u-1 at
        # s-chunk granularity. Units are the two mc halves; fillers per chunk
        # are the previous chunk's out-projection, the next chunk's Q
        # projection, and (chunks 0-1) the gate MLP.
        with ExitStack() as cctx:
            p_p = cctx.enter_context(tc.tile_pool(name="p_p", bufs=5))
            p_a = cctx.enter_context(tc.tile_pool(name="p_a", bufs=2))
            p_s = cctx.enter_context(tc.tile_pool(name="p_s", bufs=2))
            p_o = cctx.enter_context(tc.tile_pool(name="p_o", bufs=4))
            psS = cctx.enter_context(tc.tile_pool(name="psS", bufs=1, space="PSUM"))
            psPV = cctx.enter_context(tc.tile_pool(name="psPV", bufs=1, space="PSUM"))
            psO = cctx.enter_context(tc.tile_pool(name="psO", bufs=2, space="PSUM"))
            p_cx = cctx.enter_context(tc.tile_pool(name="p_cx", bufs=1))
            p_x2 = cctx.enter_context(tc.tile_pool(name="p_x2", bufs=2))

            def emit_qk_exp(mc, si, rsl, rc):
                # NOTE: both packed matmuls must target the SAME psum tile --
                # per-head psum tiles serialize the tile_position pair
                # (measured +136ns per QK pair)
                sps2 = psS.tile([SCH, 2, RC], F32, tag="sps", name="sps")
                for hh in range(2):  # row-packed pair, adjacent emission
                    po = hh * 64
                    nc.tensor.matmul(
                        sps2[:, hh, 0:rc],
                        kt_sb[po:po + 64, mc, si * SCH:(si + 1) * SCH],
                        qt_sb[po:po + 64, mc, rsl],
                        start=True, stop=True,
                        tile_position=(po, 0))
                p2 = p_p.tile([SCH, 2, RC], BF16, tag=f"p{si % 2}",
                              name=f"p{si % 2}")
                nc.scalar.activation(p2[:, :, 0:rc], sps2[:, :, 0:rc],
                                     AF.Exp, scale=0.125)
                return p2

            def emit_pv(aps, mc, si, p2, rc):
                for hh in range(2):
                    h = mc * 2 + hh
                    nc.tensor.matmul(
                        aps[hh][:, 0:rc], vx_sb[:, si, h, :],
                        p2[:, hh, 0:rc],
                        start=(si == 0), stop=(si == NSC - 1))

            def emit_norm(aps, mc, a2, rc):
                for hh in range(2):
                    den1 = p_s.tile([1, RC], F32, tag="den1")
                    nc.vector.tensor_copy(den1[:, 0:rc], aps[hh][64:65, 0:rc])
                    rec1 = p_s.tile([1, RC], F32, tag="rec1")
                    nc.vector.reciprocal_approx_fast(rec1[:, 0:rc], den1[:, 0:rc])
                    sct64 = p_s.tile([64, RC], F32, tag="sct64")
                    nc.gpsimd.partition_broadcast(sct64[:, 0:rc], rec1[:, 0:rc])
                    nc.vector.tensor_mul(
                        a2[mc][hh * 64:hh * 64 + 64, 0:rc],
                        aps[hh][0:64, 0:rc], sct64[:, 0:rc])

            bstate = {}

            def emit_q_load(row0, rc):
                xtB = p_x2.tile([128, KD, RC], BF16, tag="xB", name="xB")
                nc.gpsimd.dma_start(xtB[:, :, 0:rc],
                                    xT.ap()[:, :, row0:row0 + rc])
                bstate["x"] = xtB

            def emit_q_proj(row0, rc, mcq):
                qps = psO.tile([128, RC], F32, tag="ops", name="qpsB")
                for kc in range(KD):
                    nc.tensor.matmul(
                        qps[:, 0:rc],
                        wq_t[:, kc, mcq * 128:(mcq + 1) * 128],
                        bstate["x"][:, kc, 0:rc],
                        start=(kc == 0), stop=(kc == KD - 1))
                nc.scalar.activation(
                    qt_sb[:, mcq, row0:row0 + rc], qps[:, 0:rc], AF.Identity,
                    bias=bq_sb[:, mcq:mcq + 1])

            def emit_gate_piece(mc):
                # hidden chunk mc for this core's 512 gate rows + its logit
                # partial, accumulated in SBUF (no persistent PSUM bank)
                hps = psO.tile([128, GR], F32, tag="ops", name="hpsG")
                for kc in range(2 * KD):
                    nc.tensor.matmul(
                        hps[:],
                        w1_t[:, kc, mc * 128:(mc + 1) * 128],
                        xg_t["t" if kc < KD else "d"][:, kc % KD, :],
                        start=(kc == 0), stop=(kc == 2 * KD - 1))
                htmp = p_s.tile([128, GR], BF16, tag="htmp", name="htmp")
                nc.scalar.activation(htmp[:], hps[:], AF.Relu,
                                     bias=gb1_sb[:, mc:mc + 1])
                lps = psO.tile([128, GR], F32, tag="ops", name="lpsG")
                nc.tensor.matmul(lps[0:1, :], w2_t[:, mc, :], htmp[:],
                                 start=True, stop=True)
                lacc = bstate["lacc"]
                if mc == 0:
                    nc.vector.tensor_copy(lacc[:], lps[0:1, :])
                else:
                    nc.vector.tensor_add(lacc[:], lacc[:], lps[0:1, :])

            def emit_gate_final():
                gate_sb = p_cx.tile([1, GR], F32, tag="gate", name="gate_sb")
                nc.scalar.activation(gate_sb[:], bstate["lacc"][:],
                                     AF.Sigmoid, bias=gb2_sb[:])
                nc.sync.dma_start(gate_out.ap(), gate_sb[:])

            def emit_gate_init():
                bstate["lacc"] = p_cx.tile([1, GR], F32, tag="lacc", name="lacc")

            def make_outproj_tasks(a2p, row0c, nrb, final=False):
                """Output projection for the rows-chunk at row0c (nrb 128-row
                blocks). One task per PSUM group (2-matmul chain over the two
                mc halves); osb assembled per (rb, half) and DMA'd out."""
                state = {}

                def group(rb, ncc):
                    # spread the output bursts over all three DMA queues;
                    # (rb*2+half) keeps all 3 in play even for 2-rb chunks
                    # (rb%3 left gpsimd idle and the last 2MB drained ~10us
                    # past the final matmul)
                    row0 = row0c + rb * 128
                    half, nh = divmod(ncc, 4)
                    dq = (nc.sync, nc.scalar, nc.gpsimd)[(rb * 8 + ncc) % 3
                                                        if final
                                                        else (rb * 2 + half) % 3]
                    if nh == 0:
                        state[(rb, half)] = p_o.tile([128, DLLM // 2], BF16,
                                                     tag="osb", name="osb")
                    osb = state[(rb, half)]
                    nsl = slice(ncc * 512, (ncc + 1) * 512)
                    ops = psO.tile([128, 512], F32, tag="ops", name="ops")
                    for kk in range(2):
                        nc.tensor.matmul(
                            ops[:], a2p[kk][:, rb * 128:(rb + 1) * 128],
                            wo_t[:, kk, nsl],
                            start=(kk == 0), stop=(kk == 1))
                    if (ncc % 2 == 1) if final else (ncc % 4 == 3):
                        # scalar has some slack next to vector; offload part
                        # of the psum->sbuf copies (half once nothing but the
                        # drain remains; gpsimd cannot read PSUM)
                        nc.scalar.activation(osb[:, nh * 512:(nh + 1) * 512],
                                             ops[:], AF.Identity)
                    else:
                        nc.vector.tensor_copy(osb[:, nh * 512:(nh + 1) * 512],
                                              ops[:])
                    if final:
                        # drain the last chunk piecewise: 0.25MB per DMA,
                        # issued as soon as each pair of columns is copied
                        if nh % 2 == 1:
                            dq.dma_start(
                                out.ap()[row0:row0 + 128,
                                         (ncc - 1) * 512:(ncc + 1) * 512],
                                osb[:, (nh - 1) * 512:(nh + 1) * 512])
                    elif nh == 3:
                        dq.dma_start(
                            out.ap()[row0:row0 + 128,
                                     half * (DLLM // 2):(half + 1) * (DLLM // 2)],
                            osb[:])

                return [(lambda rb=rb, ncc=ncc: group(rb, ncc))
                        for rb in range(nrb) for ncc in range(8)]

            # last 512 rows split in two 256-row chunks so the trailing
            # (non-overlapped) out-projection of the final chunk halves
            chunks = [(r * RC, RC) for r in range(NRC - 1)]
            chunks += [((NRC - 1) * RC, RC // 2), ((NRC - 1) * RC + RC // 2, RC // 2)]
            qsteps = [(row0c, rc) for row0c, rc in chunks[1:]] + [None]
            pending = None   # (a2, row0, nrb) awaiting out projection
            for ci, (row0c, rc) in enumerate(chunks):
                rsl = slice(row0c, row0c + rc)
                a2 = [p_a.tile([128, RC], BF16, tag=f"a2_{mc}",
                               name=f"a2_{mc}") for mc in range(2)]
                # fillers: dependency-free work (next chunk's Q projection,
                # gate MLP pieces on chunks 0-1) paces from slot 0 and covers
                # the first slots; the prev chunk's out-proj paces from slot
                # 3 because its first group reads a2 written by a norm chain
                # still in flight at chunk start
                # fillers this chunk: next chunk's Q projection (dependency
                # free, so it leads), then the prev chunk's out-proj -- its
                # first group must wait out the in-flight norm of THIS
                # chunk's predecessor, hence the 3-slot pacing delay -- and
                # the gate MLP during chunks 0-1
                tasks = []
                if qsteps[ci] is not None:
                    qr0, qrc = qsteps[ci]
                    tasks += [lambda r=qr0, c=qrc: emit_q_load(r, c),
                              lambda r=qr0, c=qrc: emit_q_proj(r, c, 0),
                              lambda r=qr0, c=qrc: emit_q_proj(r, c, 1)]
                if ci == 0:
                    tasks += [emit_gate_init]
                    tasks += [(lambda m=m: emit_gate_piece(m)) for m in range(4)]
                elif ci == 1:
                    tasks += [(lambda m=m: emit_gate_piece(m)) for m in range(4, 8)]
                    tasks += [emit_gate_final]
                if pending is not None:
                    tasks += make_outproj_tasks(*pending)
                ti = 0
                slot = 0
                nslots = 3 * NSC   # 2 units + drain, at s-chunk granularity
                prev = None        # (aps, mc, p2dict)
                for mc in range(2):
                    aps = [psPV.tile([65, RC], F32, tag=f"aps{mc}{hh}",
                                     name=f"aps{mc}{hh}") for hh in range(2)]
                    p2buf = {}
                    for si in range(NSC):
                        p2buf[si] = emit_qk_exp(mc, si, rsl, rc)
                        if prev is not None:
                            paps, pmc, pp2 = prev
                            emit_pv(paps, pmc, si, pp2[si], rc)
                        slot += 1
                        due = max(0, ((slot - 3) * len(tasks)) // (nslots - 3))
                        while ti < due:
                            tasks[ti]()
                            ti += 1
                    if prev is not None:
                        emit_norm(prev[0], prev[1], a2, rc)
                    prev = (aps, mc, p2buf)
                # drain last unit of this rows-chunk; fillers keep interleaving
                paps, pmc, pp2 = prev
                for si in range(NSC):
                    emit_pv(paps, pmc, si, pp2[si], rc)
                    slot += 1
                    due = max(0, ((slot - 3) * len(tasks)) // (nslots - 3))
                    while ti < due:
                        tasks[ti]()
                        ti += 1
                emit_norm(paps, pmc, a2, rc)
                while ti < len(tasks):
                    tasks[ti]()
                    ti += 1
                pending = (a2, row0c, rc // 128)
            for task in make_outproj_tasks(*pending, final=True):
                task()
        pre1ctx.close()

    nc.compile()
    return nc


def _prep_inputs(inputs):
    """Host-side shard + transpose. Returns in_maps for 8 cores."""
    f32 = np.float32
    t = {k: np.asarray(v) for k, v in inputs.items()}
    x_full = {"t": t["trend_emb"].reshape(B * L, D).astype(f32),
              "d": t["detail_emb"].reshape(B * L, D).astype(f32)}
    pT_full = {"t": np.ascontiguousarray(t["trend_proto"].astype(f32).T).astype(bf16),
               "d": np.ascontiguousarray(t["detail_proto"].astype(f32).T).astype(bf16)}
    W = {("q", "t"): t["t_Wq"], ("q", "d"): t["d_Wq"],
         ("k", "t"): t["t_Wk"], ("k", "d"): t["d_Wk"],
         ("v", "t"): t["t_Wv"], ("v", "d"): t["d_Wv"],
         ("o", "t"): t["t_Wo"], ("o", "d"): t["d_Wo"]}
    bias = {("q", "t"): t["t_bq"], ("q", "d"): t["d_bq"],
            ("k", "t"): t["t_bk"], ("k", "d"): t["d_bk"],
            ("v", "t"): t["t_bv"], ("v", "d"): t["d_bv"]}

    def to_p(a):
        c = a.shape[0] // 128
        return np.ascontiguousarray(
            a.reshape(c, 128, a.shape[1]).transpose(1, 0, 2)).astype(bf16)

    xT_blk = {b: to_p(np.ascontiguousarray(x_full[b].T)) for b in "td"}
    wq_blk, wk_blk, wv_blk, wo_blk, bq_blk, bk_blk, bv_blk = ({} for _ in range(7))
    for b in "td":
        for hg in range(HG):
            hsl = slice(hg * HEC, (hg + 1) * HEC)
            wq_blk[b, hg] = to_p(np.ascontiguousarray(W[("q", b)][:, hsl]))
            wk_blk[b, hg] = to_p(np.ascontiguousarray(W[("k", b)][:, hsl]))
            wv_blk[b, hg] = to_p(np.ascontiguousarray(W[("v", b)][:, hsl]))
            wo_blk[b, hg] = to_p(np.ascontiguousarray(W[("o", b)][hsl, :]))
            bq_blk[b, hg] = np.stack([bias[("q", b)][hsl][0:128],
                                      bias[("q", b)][hsl][128:256]], axis=1).astype(f32)
            bk_blk[b, hg] = np.stack([bias[("k", b)][hsl][0:128],
                                      bias[("k", b)][hsl][128:256]], axis=1).astype(f32)
            bv_blk[b, hg] = bias[("v", b)][hsl][None, :].astype(bf16)
    w1_p = to_p(t["g_W1"])
    w2_p = to_p(t["g_W2"])
    gb1_p = np.ascontiguousarray(t["g_b1"].astype(f32).reshape(KD, 128).T)
    gb2_p = t["g_b2"].astype(f32).reshape(1, 1)

    in_maps = []
    for core in range(8):
        blk = "t" if core < 4 else "d"
        hg = core % HG
        grow = slice(core * GR, (core + 1) * GR)
        m = {"xT": xT_blk[blk], "pT": pT_full[blk],
             "wq": wq_blk[blk, hg], "wk": wk_blk[blk, hg],
             "wv": wv_blk[blk, hg], "wo": wo_blk[blk, hg],
             "bq2": bq_blk[blk, hg], "bk2": bk_blk[blk, hg],
             "bv": bv_blk[blk, hg],
             "w1": w1_p, "w2": w2_p, "gb1": gb1_p, "gb2": gb2_p}
        for b in "td":
            m[f"xgT_{b}"] = to_p(np.ascontiguousarray(x_full[b][grow].T))
        in_maps.append(m)
    return in_maps


def kernel(**inputs):
    global LAST_RESULTS
    import os
    from concourse.bass_utils import run_bass_kernel_spmd

    in_maps = _prep_inputs(inputs)
    if "nc" not in _CACHE:
        _CACHE["nc"] = _build()
    nc = _CACHE["nc"]

    trace = bool(os.environ.get("KERNEL_TRACE"))
    res = run_bass_kernel_spmd(
        nc, in_maps, list(range(8)),
        trace=trace, trace_cores=list(range(8)) if trace else None)
    LAST_RESULTS = res

    t = {k: np.asarray(v) for k, v in inputs.items()}
    bo = {"t": t["t_bo"].astype(np.float32), "d": t["d_bo"].astype(np.float32)}
    g = np.concatenate([res.results[c]["gate"][0]
                        for c in range(8)]).astype(np.float32)[:, None]
    acc = {}
    for bi, b in enumerate("td"):
        a = res.results[bi * HG]["out"].astype(np.float32)
        for hg in range(1, HG):
            a = a + res.results[bi * HG + hg]["out"]
        acc[b] = a + bo[b][None, :]
    out = g * acc["t"] + (1.0 - g) * acc["d"]
    return out.reshape(B, L, DLLM)
